# revision 32
# baseline (speedup 1.0000x reference)
"""CenterLoss kernel for 8 Trainium2 NeuronCores (Bass/Tile).

Reference computation (see problem):
    f = feature.transpose(0,2,1).reshape(-1, D); f = f / max(||f||, 1e-12)
    center_loss = mean((f - centers[lab])**2)
    difference  = segment_sum(centers[lab] - f) / max(counts, 1)
    returns (center_loss, difference, cls, labels)

Algebraic restructuring (exact in real arithmetic):
    S[c]      = segment_sum(f)[c]          (segment sum of *normalized* rows)
    counts[c] = bincount(lab)[c]
    difference[c] = (counts[c]*centers[c] - S[c]) / max(counts[c], 1)
    center_loss   = (R - 2*sum_c S[c].centers[c] + sum_c counts[c]*||centers[c]||^2) / (N*D)
    where R = sum_t ||f_t||^2 == N for non-degenerate rows (norm >= eps always
    holds for randn data; a row would need to be exactly zero to differ).

Device mapping (SPMD over 8 cores, 8 batches each):
    - Feature memory layout is [D, T] per batch; the segment matmul needs
      token-major tiles, so each 128-token tile is transposed on the
      TensorEngine via matmul with an identity rhs (out = lhsT.T @ I).
    - Norms: ACT squares the transposed PSUM tile, DVE reduces rows.
    - Normalize: DVE multiplies the PSUM tile by rsqrt(norm2) broadcast while
      copying to SBUF (bf16), appending a ones column for counts.
    - One-hot H[t, c] = (lab[t] == c) built by DVE is_equal against an int8
      iota block, batched 16 tiles per instruction.
    - Segment matmul accumulates [100, 129] = H^T @ [F_norm | 1] in PSUM.
    - AllReduce (add, bf16 payload) of the [100, 129] partials over the 8
      cores, then every core computes the final difference/loss identically.
    - labels are passed to the device pre-transposed into token-major columns
      (host-side layout change only) so the label DMA is contiguous.
"""

import os
import numpy as np

B, D, T, CLASSES = 64, 128, 4096, 100
N_CORES = 8
BPC = B // N_CORES

P = 128
GRP = 8              # 128-token tiles per PSUM group (2 banks per group)
GRP_TOK = P * GRP    # 1024

_NC_CACHE = {}
LAST_RESULT = None   # BassKernelResults of the most recent kernel() call


def _np_bf16():
    import ml_dtypes
    return ml_dtypes.bfloat16


def make_consts(t=T):
    """Host-side constant inputs (replicated to every core)."""
    iota = np.tile(np.arange(CLASSES, dtype=np.float32), (P, 1))     # [P, C]
    ident = np.eye(P, dtype=_np_bf16())                              # [P, P]
    half = min(16, t // P)
    iota_rep = np.tile(np.arange(CLASSES, dtype=np.int8),
                       (P, half, 1))                                 # [P, half, C]
    return iota, ident, iota_rep


def build_nc(n_cores=N_CORES, bpc=BPC, t=T, h_engine="vector"):
    """Build + compile the SPMD Bass graph. Shapes are per-core."""
    import concourse.bacc as bacc
    import concourse.tile as tile
    from concourse import mybir

    f32 = mybir.dt.float32
    bf16 = mybir.dt.bfloat16
    i32 = mybir.dt.int32
    X = mybir.AxisListType.X
    op = mybir.AluOpType

    assert t % GRP_TOK == 0
    n_tiles = t // P                    # 128-token tiles per batch
    n_groups = t // GRP_TOK             # PSUM groups per batch
    half = min(16, n_tiles)             # tiles per H-build batch
    assert n_tiles % half == 0 and half % GRP == 0
    n_half = n_tiles // half
    grp_per_half = half // GRP

    n_total = n_cores * bpc * t         # full token count over all cores
    inv_nd = 1.0 / (float(n_total) * D)

    nc = bacc.Bacc("TRN2", target_bir_lowering=False, debug=False,
                   num_devices=n_cores)

    feat = nc.dram_tensor("feature", [bpc, D, t], f32, kind="ExternalInput")
    # labels pre-transposed on the host to [P, n_tiles] column layout per
    # batch: lab[t_lo, j] = labels[b, j*128 + t_lo] — contiguous DMA here
    labels = nc.dram_tensor("labels", [bpc, P, t // P], i32,
                            kind="ExternalInput")
    centers = nc.dram_tensor("centers", [CLASSES, D], f32, kind="ExternalInput")
    i8 = mybir.dt.int8
    iota_in = nc.dram_tensor("iota_c", [P, CLASSES], f32, kind="ExternalInput")
    ident_in = nc.dram_tensor("ident_bf16", [P, P], bf16, kind="ExternalInput")
    iota_rep_in = nc.dram_tensor("iota_rep", [P, half, CLASSES], i8,
                                 kind="ExternalInput")
    loss_out = nc.dram_tensor("loss", [1, 1], f32, kind="ExternalOutput")
    diff_out = nc.dram_tensor("difference", [CLASSES, D], f32,
                              kind="ExternalOutput")

    with tile.TileContext(nc) as tc:
        with (
            tc.tile_pool(name="consts", bufs=1) as consts,
            tc.tile_pool(name="slab", bufs=2) as slab_pool,
            tc.tile_pool(name="lab", bufs=2) as lab_pool,
            tc.tile_pool(name="work", bufs=3) as work,
            tc.tile_pool(name="rhsp", bufs=4) as rhs_pool,
            tc.tile_pool(name="hpool", bufs=3) as hpool,
            tc.tile_pool(name="small", bufs=6) as small,
            tc.tile_pool(name="fin", bufs=1) as fin,
            tc.tile_pool(name="psg", bufs=3, space="PSUM") as psg,
            tc.tile_pool(name="pss", bufs=1, space="PSUM") as pss,
            tc.tile_pool(name="psf", bufs=1, space="PSUM") as psf,
            tc.tile_pool(name="dramp", bufs=1, space="DRAM") as drp,
        ):
            ident = consts.tile([P, P], bf16)
            nc.sync.dma_start(out=ident[:], in_=ident_in[:])
            iota = consts.tile([P, CLASSES], f32)
            nc.sync.dma_start(out=iota[:], in_=iota_in[:])
            iota_rep = consts.tile([P, half, CLASSES], i8)
            nc.sync.dma_start(out=iota_rep[:], in_=iota_rep_in[:])

            # centers needed only for the epilogue, but load early (DMA is idle)
            cent = fin.tile([CLASSES, D], f32)
            nc.sync.dma_start(out=cent[:], in_=centers[:])

            psum_S = pss.tile([CLASSES, D + 1], f32)

            # scale-copies split between scalar engine (per tile, ~527ns)
            # and vector engine (per group, ~1050ns); alternate 2/1 per half
            # for an effective 1.5/4 on ACT

            mm_idx = 0
            total_mms = bpc * n_tiles
            n_chunks = 4 if t >= 4096 else 1
            ch_t = t // n_chunks
            for b in range(bpc):
                # cast-load (f32 -> bf16 in the DMA, SWDGE) in chunks, each its
                # own tile so downstream deps are per-chunk
                chunks = []
                for ci in range(n_chunks):
                    ch = slab_pool.tile([P, ch_t], bf16, tag=f"slab{ci}")
                    chunks.append(ch)
                    nc.gpsimd.dma_start(
                        out=ch[:],
                        in_=feat[b, :, ci * ch_t:(ci + 1) * ch_t])

                def slab_slice(ti):
                    ci, loc = divmod(ti * P, ch_t)
                    return chunks[ci][:, loc:loc + P]

                lab_i = lab_pool.tile([P, n_tiles], i32)
                nc.sync.dma_start(out=lab_i[:], in_=labels[b])
                lab_f = lab_pool.tile([P, n_tiles], i8)
                nc.vector.tensor_copy(out=lab_f[:], in_=lab_i[:])

                for hh in range(n_half):
                    tile_base = hh * half
                    # one-hot H for `half` tiles at once: H[t, j, c] = (lab == c)
                    H_half = hpool.tile([P, half, CLASSES], bf16)
                    nc.vector.tensor_tensor(
                        out=H_half[:],
                        in0=iota_rep[:],
                        in1=lab_f[:, tile_base:tile_base + half, None]
                            .to_broadcast([P, half, CLASSES]),
                        op=op.is_equal,
                    )

                    # --- phase 1: transposes, square (ACT), row-sums (DVE) ---
                    psums = []
                    nsp = small.tile([P, half], f32)
                    for gg in range(grp_per_half):
                        tile0 = tile_base + gg * GRP
                        psum_g = psg.tile([P, GRP_TOK], f32)
                        psums.append(psum_g)
                        for i in range(GRP):
                            ti = tile0 + i
                            nc.tensor.matmul(
                                out=psum_g[:, i * P:(i + 1) * P],
                                lhsT=slab_slice(ti),
                                rhs=ident[:],
                                start=True, stop=True,
                                skip_group_check=True,
                            )
                        sq = work.tile([P, GRP_TOK], bf16)
                        nc.scalar.activation(
                            out=sq[:], in_=psum_g[:],
                            func=mybir.ActivationFunctionType.Square,
                        )
                        nc.vector.tensor_reduce(
                            out=nsp[:, gg * GRP:(gg + 1) * GRP],
                            in_=sq[:].rearrange("p (g d) -> p g d", g=GRP),
                            axis=X, op=op.add,
                        )

                    # --- phase 2: batched inv = rsqrt(max(ns, eps^2)) ---
                    nsc = small.tile([P, half], f32)
                    nc.vector.tensor_scalar_max(out=nsc[:], in0=nsp[:],
                                                scalar1=1e-24)
                    rec = small.tile([P, half], f32)
                    nc.vector.reciprocal(out=rec[:], in_=nsc[:])
                    inv = small.tile([P, half], f32)
                    nc.scalar.sqrt(out=inv[:], in_=rec[:])

                    # --- phase 3: normalize + ones col + segment matmuls ---
                    act_scale_groups = 1
                    for gg in range(grp_per_half):
                        psum_g = psums[gg]
                        rhs_g = rhs_pool.tile([P, GRP, D + 1], bf16)
                        if gg < act_scale_groups:
                            # scalar engine: per-tile copy with AP scale
                            for i in range(GRP):
                                j = gg * GRP + i
                                nc.scalar.activation(
                                    out=rhs_g[:, i, 0:D],
                                    in_=psum_g[:, i * P:(i + 1) * P],
                                    func=mybir.ActivationFunctionType.Copy,
                                    scale=inv[:, j:j + 1],
                                )
                        else:
                            nc.vector.tensor_tensor(
                                out=rhs_g[:, :, 0:D],
                                in0=psum_g[:].rearrange("p (g d) -> p g d",
                                                        g=GRP),
                                in1=inv[:, gg * GRP:(gg + 1) * GRP, None]
                                    .to_broadcast([P, GRP, D]),
                                op=op.mult,
                            )
                        nc.gpsimd.memset(rhs_g[:, :, D:D + 1], 1.0)
                        for i in range(GRP):
                            mm_idx += 1
                            nc.tensor.matmul(
                                out=psum_S[:],
                                lhsT=H_half[:, gg * GRP + i, :],
                                rhs=rhs_g[:, i, :],
                                start=(mm_idx == 1),
                                stop=(mm_idx == total_mms),
                                skip_group_check=True,
                            )

            # ---- AllReduce of [S | counts] over the cores ----
            # bf16 payload (halves the wire bytes); rows padded to 144 bf16 =
            # 288B (32B-aligned). Counts ~328/core round at ~0.2% in bf16 --
            # well inside the accuracy gate.
            AR_W = 144
            sb_S = fin.tile([CLASSES, AR_W], bf16)
            nc.vector.memset(sb_S[:, D + 1:AR_W], 0.0)
            nc.vector.tensor_copy(out=sb_S[:, 0:D + 1], in_=psum_S[:])
            ar_in = drp.tile([CLASSES, AR_W], bf16)
            ar_out = drp.tile([CLASSES, AR_W], bf16)
            nc.sync.dma_start(out=ar_in[:], in_=sb_S[:])
            nc.gpsimd.collective_compute(
                "AllReduce", op.add,
                replica_groups=[list(range(n_cores))],
                ins=[ar_in[:].opt()],
                outs=[ar_out[:].opt()],
            )
            res_b = fin.tile([CLASSES, AR_W], bf16)
            nc.sync.dma_start(out=res_b[:], in_=ar_out[:])
            res = fin.tile([CLASSES, D + 1], f32)
            nc.vector.tensor_copy(out=res[:], in_=res_b[:, 0:D + 1])

            # ---- epilogue: difference + loss (identical on every core) ----
            S_ap = res[:, 0:D]
            cnt = res[:, D:D + 1]
            cntc = fin.tile([CLASSES, 1], f32)
            nc.vector.tensor_scalar_max(out=cntc[:], in0=cnt, scalar1=1.0)
            rcnt = fin.tile([CLASSES, 1], f32)
            nc.vector.reciprocal(out=rcnt[:], in_=cntc[:])

            num = fin.tile([CLASSES, D], f32)
            # num = centers*counts - S
            nc.vector.scalar_tensor_tensor(
                out=num[:], in0=cent[:], scalar=cnt, in1=S_ap,
                op0=op.mult, op1=op.subtract,
            )
            diff = fin.tile([CLASSES, D], f32)
            nc.vector.tensor_scalar_mul(out=diff[:], in0=num[:],
                                        scalar1=rcnt[:])
            nc.sync.dma_start(out=diff_out[:], in_=diff[:])

            # loss pieces: W[c] = S[c].centers[c], cn2[c] = ||centers[c]||^2
            W = fin.tile([CLASSES, 1], f32)
            tmpW = fin.tile([CLASSES, D], f32)
            nc.vector.scalar_tensor_tensor(
                out=tmpW[:], in0=S_ap, scalar=1.0, in1=cent[:],
                op0=op.bypass, op1=op.mult, accum_out=W[:],
            )
            cn2 = fin.tile([CLASSES, 1], f32)
            tmpC = fin.tile([CLASSES, D], f32)
            nc.vector.scalar_tensor_tensor(
                out=tmpC[:], in0=cent[:], scalar=1.0, in1=cent[:],
                op0=op.bypass, op1=op.mult, accum_out=cn2[:],
            )
            V = fin.tile([CLASSES, 1], f32)
            nc.vector.tensor_mul(out=V[:], in0=cnt, in1=cn2[:])
            lv = fin.tile([CLASSES, 1], f32)
            # lv = -2*W + V
            nc.vector.scalar_tensor_tensor(
                out=lv[:], in0=W[:], scalar=-2.0, in1=V[:],
                op0=op.mult, op1=op.add,
            )
            ones_cls = fin.tile([CLASSES, 1], f32)
            nc.vector.memset(ones_cls[:], 1.0)
            loss_ps = psf.tile([1, 1], f32)
            nc.tensor.matmul(out=loss_ps[:], lhsT=ones_cls[:], rhs=lv[:],
                             start=True, stop=True, skip_group_check=True)
            loss_sb = fin.tile([1, 1], f32)
            # loss = (sum(lv) + N) / (N*D)  ->  sum(lv)*inv_nd + 1/D
            nc.scalar.activation(
                out=loss_sb[:], in_=loss_ps[:],
                func=mybir.ActivationFunctionType.Copy,
                bias=1.0 / D, scale=inv_nd,
            )
            nc.sync.dma_start(out=loss_out[:], in_=loss_sb[:])

    nc.compile()
    return nc


def _get_nc():
    key = (N_CORES, BPC, T)
    if key not in _NC_CACHE:
        _NC_CACHE[key] = build_nc(*key)
    return _NC_CACHE[key]


def make_in_maps(feature, labels, centers, n_cores=N_CORES, bpc=None):
    if bpc is None:
        bpc = feature.shape[0] // n_cores
    t = feature.shape[2]
    iota, ident, iota_rep = make_consts(t)
    feature = np.ascontiguousarray(feature, dtype=np.float32)
    labels = np.ascontiguousarray(labels, dtype=np.int32)
    centers = np.ascontiguousarray(centers, dtype=np.float32)
    n_tiles = t // P
    # [bpc, t] -> [bpc, P, n_tiles] with lab[b, t_lo, j] = labels[b, j*P + t_lo]
    labels_dev = np.ascontiguousarray(
        labels.reshape(-1, n_tiles, P).transpose(0, 2, 1))
    return [
        {
            "feature": feature[k * bpc:(k + 1) * bpc],
            "labels": labels_dev[k * bpc:(k + 1) * bpc],
            "centers": centers,
            "iota_c": iota,
            "ident_bf16": ident,
            "iota_rep": iota_rep,
        }
        for k in range(n_cores)
    ]


def kernel(feature, cls, centers, labels):
    """Full inputs in, full outputs out. Distributes over 8 NeuronCores."""
    global LAST_RESULT
    from concourse.bass_utils import run_bass_kernel_spmd

    feature = np.asarray(feature)
    centers_np = np.asarray(centers)
    labels_np = np.asarray(labels)

    nc = _get_nc()
    in_maps = make_in_maps(feature, labels_np, centers_np)
    spmd_kwargs = {}
    td = os.environ.get("KERNEL_TMPDIR")
    if td:
        spmd_kwargs["tmpdir"] = td
    res = run_bass_kernel_spmd(nc, in_maps, core_ids=list(range(N_CORES)),
                               **spmd_kwargs)
    LAST_RESULT = res
    out0 = res.results[0]
    loss = np.float32(out0["loss"].reshape(())[()])
    difference = np.asarray(out0["difference"], dtype=np.float32)
    return (loss, difference, np.asarray(cls), labels_np)


# revision 33
# speedup vs baseline: 1.3374x; 1.3374x over previous
"""CenterLoss kernel for 8 Trainium2 NeuronCores (Bass/Tile).

Reference computation (see problem):
    f = feature.transpose(0,2,1).reshape(-1, D); f = f / max(||f||, 1e-12)
    center_loss = mean((f - centers[lab])**2)
    difference  = segment_sum(centers[lab] - f) / max(counts, 1)
    returns (center_loss, difference, cls, labels)

Algebraic restructuring (exact in real arithmetic):
    S[c]      = segment_sum(f)[c]          (segment sum of *normalized* rows)
    counts[c] = bincount(lab)[c]
    difference[c] = (counts[c]*centers[c] - S[c]) / max(counts[c], 1)
    center_loss   = (R - 2*sum_c S[c].centers[c] + sum_c counts[c]*||centers[c]||^2) / (N*D)
    where R = sum_t ||f_t||^2 == N for non-degenerate rows (norm >= eps always
    holds for randn data; a row would need to be exactly zero to differ).

Device mapping (SPMD over 8 cores, 8 batches each):
    - Feature memory layout is [D, T] per batch; the segment matmul needs
      token-major tiles, so each 128-token tile is transposed on the
      TensorEngine via matmul with an identity rhs (out = lhsT.T @ I).
    - Norms: ACT squares the transposed PSUM tile, DVE reduces rows.
    - Normalize: DVE multiplies the PSUM tile by rsqrt(norm2) broadcast while
      copying to SBUF (bf16), appending a ones column for counts.
    - One-hot H[t, c] = (lab[t] == c) built by DVE is_equal against an int8
      iota block, batched 16 tiles per instruction.
    - Segment matmul accumulates [100, 129] = H^T @ [F_norm | 1] in PSUM.
    - AllReduce (add, bf16 payload) of the [100, 129] partials over the 8
      cores, then every core computes the final difference/loss identically.
    - labels are passed to the device pre-transposed into token-major columns
      (host-side layout change only) so the label DMA is contiguous.
"""

import os
import numpy as np

B, D, T, CLASSES = 64, 128, 4096, 100
N_CORES = 8
BPC = B // N_CORES

P = 128
GRP = 4              # 128-token tiles per PSUM-bank group
GRP_TOK = P * GRP    # 512

_NC_CACHE = {}
LAST_RESULT = None   # BassKernelResults of the most recent kernel() call


def _np_bf16():
    import ml_dtypes
    return ml_dtypes.bfloat16


def make_consts(t=T):
    """Host-side constant inputs (replicated to every core)."""
    iota = np.tile(np.arange(CLASSES, dtype=np.float32), (P, 1))     # [P, C]
    ident = np.eye(P, dtype=_np_bf16())                              # [P, P]
    half = min(16, t // P)
    iota_rep = np.tile(np.arange(CLASSES, dtype=np.int8),
                       (P, half, 1))                                 # [P, half, C]
    return iota, ident, iota_rep


def build_nc(n_cores=N_CORES, bpc=BPC, t=T, h_engine="vector"):
    """Build + compile the SPMD Bass graph. Shapes are per-core."""
    import concourse.bacc as bacc
    import concourse.tile as tile
    from concourse import mybir

    f32 = mybir.dt.float32
    bf16 = mybir.dt.bfloat16
    i32 = mybir.dt.int32
    X = mybir.AxisListType.X
    op = mybir.AluOpType

    assert t % GRP_TOK == 0
    n_tiles = t // P                    # 128-token tiles per batch
    n_groups = t // GRP_TOK             # PSUM groups per batch
    half = min(16, n_tiles)             # tiles per H-build batch
    assert n_tiles % half == 0 and half % GRP == 0
    n_half = n_tiles // half
    grp_per_half = half // GRP

    n_total = n_cores * bpc * t         # full token count over all cores
    inv_nd = 1.0 / (float(n_total) * D)

    nc = bacc.Bacc("TRN2", target_bir_lowering=False, debug=False,
                   num_devices=n_cores)

    feat = nc.dram_tensor("feature", [bpc, D, t], f32, kind="ExternalInput")
    # labels pre-transposed on the host to [P, n_tiles] column layout per
    # batch: lab[t_lo, j] = labels[b, j*128 + t_lo] — contiguous DMA here
    labels = nc.dram_tensor("labels", [bpc, P, t // P], i32,
                            kind="ExternalInput")
    centers = nc.dram_tensor("centers", [CLASSES, D], f32, kind="ExternalInput")
    i8 = mybir.dt.int8
    iota_in = nc.dram_tensor("iota_c", [P, CLASSES], f32, kind="ExternalInput")
    ident_in = nc.dram_tensor("ident_bf16", [P, P], bf16, kind="ExternalInput")
    iota_rep_in = nc.dram_tensor("iota_rep", [P, half, CLASSES], i8,
                                 kind="ExternalInput")
    loss_out = nc.dram_tensor("loss", [1, 1], f32, kind="ExternalOutput")
    diff_out = nc.dram_tensor("difference", [CLASSES, D], f32,
                              kind="ExternalOutput")

    with tile.TileContext(nc) as tc:
        with (
            tc.tile_pool(name="consts", bufs=1) as consts,
            tc.tile_pool(name="slab", bufs=2) as slab_pool,
            tc.tile_pool(name="lab", bufs=2) as lab_pool,
            tc.tile_pool(name="work", bufs=3) as work,
            tc.tile_pool(name="rhsp", bufs=4) as rhs_pool,
            tc.tile_pool(name="hpool", bufs=3) as hpool,
            tc.tile_pool(name="small", bufs=6) as small,
            tc.tile_pool(name="fin", bufs=1) as fin,
            tc.tile_pool(name="psg", bufs=6, space="PSUM") as psg,
            tc.tile_pool(name="pss", bufs=1, space="PSUM") as pss,
            tc.tile_pool(name="psf", bufs=1, space="PSUM") as psf,
            tc.tile_pool(name="dramp", bufs=1, space="DRAM") as drp,
        ):
            ident = consts.tile([P, P], bf16)
            nc.sync.dma_start(out=ident[:], in_=ident_in[:])
            iota = consts.tile([P, CLASSES], f32)
            nc.sync.dma_start(out=iota[:], in_=iota_in[:])
            iota_rep = consts.tile([P, half, CLASSES], i8)
            nc.sync.dma_start(out=iota_rep[:], in_=iota_rep_in[:])

            # centers needed only for the epilogue, but load early (DMA is idle)
            cent = fin.tile([CLASSES, D], f32)
            nc.sync.dma_start(out=cent[:], in_=centers[:])

            psum_S = pss.tile([CLASSES, D + 1], f32)

            # scale-copies split between scalar engine (per tile, ~527ns)
            # and vector engine (per group, ~1050ns); alternate 2/1 per half
            # for an effective 1.5/4 on ACT

            mm_idx = 0
            total_mms = bpc * n_tiles
            n_chunks = 4 if t >= 4096 else 1
            ch_t = t // n_chunks
            for b in range(bpc):
                # cast-load (f32 -> bf16 in the DMA, SWDGE) in chunks, each its
                # own tile so downstream deps are per-chunk
                chunks = []
                for ci in range(n_chunks):
                    ch = slab_pool.tile([P, ch_t], bf16, tag=f"slab{ci}")
                    chunks.append(ch)
                    nc.gpsimd.dma_start(
                        out=ch[:],
                        in_=feat[b, :, ci * ch_t:(ci + 1) * ch_t])

                def slab_slice(ti):
                    ci, loc = divmod(ti * P, ch_t)
                    return chunks[ci][:, loc:loc + P]

                lab_i = lab_pool.tile([P, n_tiles], i32)
                nc.sync.dma_start(out=lab_i[:], in_=labels[b])
                lab_f = lab_pool.tile([P, n_tiles], i8)
                nc.vector.tensor_copy(out=lab_f[:], in_=lab_i[:])

                for hh in range(n_half):
                    tile_base = hh * half
                    # one-hot H for `half` tiles at once: H[t, j, c] = (lab == c)
                    H_half = hpool.tile([P, half, CLASSES], bf16)
                    nc.vector.tensor_tensor(
                        out=H_half[:],
                        in0=iota_rep[:],
                        in1=lab_f[:, tile_base:tile_base + half, None]
                            .to_broadcast([P, half, CLASSES]),
                        op=op.is_equal,
                    )

                    # --- phase 1: transposes, square (ACT), row-sums (DVE) ---
                    psums = []
                    nsp = small.tile([P, half], f32)
                    for gg in range(grp_per_half):
                        tile0 = tile_base + gg * GRP
                        psum_g = psg.tile([P, GRP_TOK], f32)
                        psums.append(psum_g)
                        for i in range(GRP):
                            ti = tile0 + i
                            nc.tensor.matmul(
                                out=psum_g[:, i * P:(i + 1) * P],
                                lhsT=slab_slice(ti),
                                rhs=ident[:],
                                start=True, stop=True,
                                skip_group_check=True,
                            )
                        sq = work.tile([P, GRP_TOK], bf16)
                        nc.scalar.activation(
                            out=sq[:], in_=psum_g[:],
                            func=mybir.ActivationFunctionType.Square,
                        )
                        nc.vector.tensor_reduce(
                            out=nsp[:, gg * GRP:(gg + 1) * GRP],
                            in_=sq[:].rearrange("p (g d) -> p g d", g=GRP),
                            axis=X, op=op.add,
                        )

                    # --- phase 2: batched inv = rsqrt(max(ns, eps^2)) ---
                    nsc = small.tile([P, half], f32)
                    nc.vector.tensor_scalar_max(out=nsc[:], in0=nsp[:],
                                                scalar1=1e-24)
                    rec = small.tile([P, half], f32)
                    nc.vector.reciprocal(out=rec[:], in_=nsc[:])
                    inv = small.tile([P, half], f32)
                    nc.scalar.sqrt(out=inv[:], in_=rec[:])

                    # --- phase 3: normalize + ones col + segment matmuls ---
                    act_scale_groups = 2 if hh % 2 == 0 else 1
                    for gg in range(grp_per_half):
                        psum_g = psums[gg]
                        rhs_g = rhs_pool.tile([P, GRP, D + 1], bf16)
                        if gg < act_scale_groups:
                            # scalar engine: per-tile copy with AP scale
                            for i in range(GRP):
                                j = gg * GRP + i
                                nc.scalar.activation(
                                    out=rhs_g[:, i, 0:D],
                                    in_=psum_g[:, i * P:(i + 1) * P],
                                    func=mybir.ActivationFunctionType.Copy,
                                    scale=inv[:, j:j + 1],
                                )
                        else:
                            nc.vector.tensor_tensor(
                                out=rhs_g[:, :, 0:D],
                                in0=psum_g[:].rearrange("p (g d) -> p g d",
                                                        g=GRP),
                                in1=inv[:, gg * GRP:(gg + 1) * GRP, None]
                                    .to_broadcast([P, GRP, D]),
                                op=op.mult,
                            )
                        nc.gpsimd.memset(rhs_g[:, :, D:D + 1], 1.0)
                        for i in range(GRP):
                            mm_idx += 1
                            nc.tensor.matmul(
                                out=psum_S[:],
                                lhsT=H_half[:, gg * GRP + i, :],
                                rhs=rhs_g[:, i, :],
                                start=(mm_idx == 1),
                                stop=(mm_idx == total_mms),
                                skip_group_check=True,
                            )

            # ---- AllReduce of [S | counts] over the cores ----
            # bf16 payload (halves the wire bytes); rows padded to 144 bf16 =
            # 288B (32B-aligned). Counts ~328/core round at ~0.2% in bf16 --
            # well inside the accuracy gate.
            AR_W = 144
            sb_S = fin.tile([CLASSES, AR_W], bf16)
            nc.vector.memset(sb_S[:, D + 1:AR_W], 0.0)
            nc.vector.tensor_copy(out=sb_S[:, 0:D + 1], in_=psum_S[:])
            ar_in = drp.tile([CLASSES, AR_W], bf16)
            ar_out = drp.tile([CLASSES, AR_W], bf16)
            nc.sync.dma_start(out=ar_in[:], in_=sb_S[:])
            nc.gpsimd.collective_compute(
                "AllReduce", op.add,
                replica_groups=[list(range(n_cores))],
                ins=[ar_in[:].opt()],
                outs=[ar_out[:].opt()],
            )
            res_b = fin.tile([CLASSES, AR_W], bf16)
            nc.sync.dma_start(out=res_b[:], in_=ar_out[:])
            res = fin.tile([CLASSES, D + 1], f32)
            nc.vector.tensor_copy(out=res[:], in_=res_b[:, 0:D + 1])

            # ---- epilogue: difference + loss (identical on every core) ----
            S_ap = res[:, 0:D]
            cnt = res[:, D:D + 1]
            cntc = fin.tile([CLASSES, 1], f32)
            nc.vector.tensor_scalar_max(out=cntc[:], in0=cnt, scalar1=1.0)
            rcnt = fin.tile([CLASSES, 1], f32)
            nc.vector.reciprocal(out=rcnt[:], in_=cntc[:])

            num = fin.tile([CLASSES, D], f32)
            # num = centers*counts - S
            nc.vector.scalar_tensor_tensor(
                out=num[:], in0=cent[:], scalar=cnt, in1=S_ap,
                op0=op.mult, op1=op.subtract,
            )
            diff = fin.tile([CLASSES, D], f32)
            nc.vector.tensor_scalar_mul(out=diff[:], in0=num[:],
                                        scalar1=rcnt[:])
            nc.sync.dma_start(out=diff_out[:], in_=diff[:])

            # loss pieces: W[c] = S[c].centers[c], cn2[c] = ||centers[c]||^2
            W = fin.tile([CLASSES, 1], f32)
            tmpW = fin.tile([CLASSES, D], f32)
            nc.vector.scalar_tensor_tensor(
                out=tmpW[:], in0=S_ap, scalar=1.0, in1=cent[:],
                op0=op.bypass, op1=op.mult, accum_out=W[:],
            )
            cn2 = fin.tile([CLASSES, 1], f32)
            tmpC = fin.tile([CLASSES, D], f32)
            nc.vector.scalar_tensor_tensor(
                out=tmpC[:], in0=cent[:], scalar=1.0, in1=cent[:],
                op0=op.bypass, op1=op.mult, accum_out=cn2[:],
            )
            V = fin.tile([CLASSES, 1], f32)
            nc.vector.tensor_mul(out=V[:], in0=cnt, in1=cn2[:])
            lv = fin.tile([CLASSES, 1], f32)
            # lv = -2*W + V
            nc.vector.scalar_tensor_tensor(
                out=lv[:], in0=W[:], scalar=-2.0, in1=V[:],
                op0=op.mult, op1=op.add,
            )
            ones_cls = fin.tile([CLASSES, 1], f32)
            nc.vector.memset(ones_cls[:], 1.0)
            loss_ps = psf.tile([1, 1], f32)
            nc.tensor.matmul(out=loss_ps[:], lhsT=ones_cls[:], rhs=lv[:],
                             start=True, stop=True, skip_group_check=True)
            loss_sb = fin.tile([1, 1], f32)
            # loss = (sum(lv) + N) / (N*D)  ->  sum(lv)*inv_nd + 1/D
            nc.scalar.activation(
                out=loss_sb[:], in_=loss_ps[:],
                func=mybir.ActivationFunctionType.Copy,
                bias=1.0 / D, scale=inv_nd,
            )
            nc.sync.dma_start(out=loss_out[:], in_=loss_sb[:])

    nc.compile()
    return nc


def _get_nc():
    key = (N_CORES, BPC, T)
    if key not in _NC_CACHE:
        _NC_CACHE[key] = build_nc(*key)
    return _NC_CACHE[key]


def make_in_maps(feature, labels, centers, n_cores=N_CORES, bpc=None):
    if bpc is None:
        bpc = feature.shape[0] // n_cores
    t = feature.shape[2]
    iota, ident, iota_rep = make_consts(t)
    feature = np.ascontiguousarray(feature, dtype=np.float32)
    labels = np.ascontiguousarray(labels, dtype=np.int32)
    centers = np.ascontiguousarray(centers, dtype=np.float32)
    n_tiles = t // P
    # [bpc, t] -> [bpc, P, n_tiles] with lab[b, t_lo, j] = labels[b, j*P + t_lo]
    labels_dev = np.ascontiguousarray(
        labels.reshape(-1, n_tiles, P).transpose(0, 2, 1))
    return [
        {
            "feature": feature[k * bpc:(k + 1) * bpc],
            "labels": labels_dev[k * bpc:(k + 1) * bpc],
            "centers": centers,
            "iota_c": iota,
            "ident_bf16": ident,
            "iota_rep": iota_rep,
        }
        for k in range(n_cores)
    ]


def kernel(feature, cls, centers, labels):
    """Full inputs in, full outputs out. Distributes over 8 NeuronCores."""
    global LAST_RESULT
    from concourse.bass_utils import run_bass_kernel_spmd

    feature = np.asarray(feature)
    centers_np = np.asarray(centers)
    labels_np = np.asarray(labels)

    nc = _get_nc()
    in_maps = make_in_maps(feature, labels_np, centers_np)
    spmd_kwargs = {}
    td = os.environ.get("KERNEL_TMPDIR")
    if td:
        spmd_kwargs["tmpdir"] = td
    res = run_bass_kernel_spmd(nc, in_maps, core_ids=list(range(N_CORES)),
                               **spmd_kwargs)
    LAST_RESULT = res
    out0 = res.results[0]
    loss = np.float32(out0["loss"].reshape(())[()])
    difference = np.asarray(out0["difference"], dtype=np.float32)
    return (loss, difference, np.asarray(cls), labels_np)
